# revision 36
# baseline (speedup 1.0000x reference)
"""DTCWT 3-level inverse on 8 Trainium2 NeuronCores.

Every filtering stage is a banded matmul on the tensor engine in fp16
(PSUM accumulates fp32; ~7e-4 total rel err vs the 2e-2 gate).

All stages use "data as lhsT" mode: matmul(out, lhsT=data[K=h, M=w],
rhs=mat[K=h, N=h_out]) contracts over the partition dim of the data and
yields the filtered image TRANSPOSED ([w, h_out]); column and row stages
then alternate orientation naturally with zero explicit transposes.

The c2q band construction is folded into the matrices; at L1 the lowpass
path is additionally merged into the band polyphase layout ([E|O] w-planes)
so the final row stage is 4 accumulation passes instead of 6.

Schedule: phase-major (L3 x16, L2 x16, L1 x16) with double/triple-buffered
PSUM pools so neighbouring images' matmuls hide each other's copy latency.
DMA queue slots cost ~600ns regardless of size, so all loads are batched
into a handful of giant multi-dim DMAs (2 matrix blobs, 9 input sweeps,
1 store per image).

Sharding: pure data parallel over batch N (8 cores x 16 channels each).
"""
import sys

for _p in ('/opt/trn_rl_repo',):
    if _p not in sys.path:
        sys.path.append(_p)

import numpy as np
import concourse.bass as bass
import concourse.mybir as mybir
from concourse.tile import TileContext
from concourse.bass_utils import run_bass_kernel_spmd

SQRT_HALF = 0.7071067811865476
N_CORES = 8
IMGS_PER_CORE = 16
F32 = mybir.dt.float32
F16 = mybir.dt.float16


# ---------------------------------------------------------------------------
# Host-side matrix construction (numpy, float64)
# ---------------------------------------------------------------------------
def _conv_rows_valid(x, h):
    hr = h[::-1]
    taps = h.shape[0]
    n = x.shape[-2] - taps + 1
    out = hr[0] * x[..., 0:n, :]
    for k in range(1, taps):
        out = out + hr[k] * x[..., k:k + n, :]
    return out


def _pad_rows_symmetric(x, m):
    pad = [(0, 0)] * (x.ndim - 2) + [(m, m), (0, 0)]
    return np.pad(x, pad, mode='symmetric')


def _colfilter(x, h):
    return _conv_rows_valid(_pad_rows_symmetric(x, h.shape[0] // 2), h)


def _colifilt(x, ha, hb, highpass):
    m = ha.shape[0]
    m2 = m // 2
    r = x.shape[-2]
    xp = _pad_rows_symmetric(x, m2)
    xe = xp[..., 1:r + m - 2:2, :]
    xo = xp[..., 2:r + m - 1:2, :]
    xa, xb = (xe, xo) if highpass else (xo, xe)
    hao, hae = ha[0::2], ha[1::2]
    hbo, hbe = hb[0::2], hb[1::2]
    y0 = _conv_rows_valid(xb, hao)
    y1 = _conv_rows_valid(xa, hbo)
    y2 = _conv_rows_valid(xb, hae)
    y3 = _conv_rows_valid(xa, hbe)
    y = np.stack([y0, y1, y2, y3], axis=-2)
    return y.reshape(y.shape[:-3] + (2 * r, y.shape[-1]))


def _op_matrix(op, n):
    """M[h_in, h_out] with out[h_out, w] = sum_h M[h, h_out] x[h, w]."""
    return np.ascontiguousarray(op(np.eye(n, dtype=np.float64)).T)


def build_matrices(g0o, g1o, g0a, g0b, g1a, g1b):
    """All device matrices as {name: fp16 ndarray}."""
    g0o = np.asarray(g0o, np.float64)
    g1o = np.asarray(g1o, np.float64)
    g0a = np.asarray(g0a, np.float64)
    g0b = np.asarray(g0b, np.float64)
    g1a = np.asarray(g1a, np.float64)
    g1b = np.asarray(g1b, np.float64)
    s = SQRT_HALF
    hs, vs = np.hstack, np.vstack
    out = {}

    def upsample_level(R, tag):
        Mlo = _op_matrix(lambda x: _colifilt(x, g0b, g0a, False), R)  # [R, 2R]
        Mhi = _op_matrix(lambda x: _colifilt(x, g1b, g1a, True), R)
        Me_h, Mo_h = s * Mhi[0::2], s * Mhi[1::2]                     # [R/2, 2R]
        Me_l, Mo_l = s * Mlo[0::2], s * Mlo[1::2]
        out[f'M{tag}_lo'] = Mlo
        # pair-stacked [w1; w2] col rhs, e|o column-concatenated
        #   e: w1r*Me + w2r*Me + w1i*Mo - w2i*Mo
        #   o: -w1r*Mo + w2r*Mo + w1i*Me + w2i*Me
        out[f'L{tag}_hi_R'] = hs([vs([Me_h, Me_h]), vs([-Mo_h, Mo_h])])
        out[f'L{tag}_hi_I'] = hs([vs([Mo_h, -Mo_h]), vs([Me_h, Me_h])])
        out[f'L{tag}_lo_R'] = hs([vs([Me_l, Me_l]), vs([-Mo_l, Mo_l])])
        out[f'L{tag}_lo_I'] = hs([vs([Mo_l, -Mo_l]), vs([Me_l, Me_l])])
        # row stage (polyphase-column recombination)
        out[f'Be{tag}_lo'], out[f'Bo{tag}_lo'] = Mlo[0::2], Mlo[1::2]
        out[f'Be{tag}_hi'], out[f'Bo{tag}_hi'] = Mhi[0::2], Mhi[1::2]

    upsample_level(64, '3')
    upsample_level(128, '2')
    # L3 quad stacks: [hl pair (lo mats); hh pair (hi mats)], K=128
    out['L3_q_R'] = vs([out['L3_lo_R'], out['L3_hi_R']])
    out['L3_q_I'] = vs([out['L3_lo_I'], out['L3_hi_I']])
    del out['L3_lo_R'], out['L3_lo_I']  # only used inside the quad at L3

    # L1 (colfilter, size-preserving, n=256)
    A_lo = _op_matrix(lambda x: _colfilter(x, g0o), 256)              # [256, 256]
    A_hi = _op_matrix(lambda x: _colfilter(x, g1o), 256)
    out['Alo_a'], out['Alo_b'] = A_lo[0:128], A_lo[128:256]
    for x, A in (('hi', A_hi), ('lo', A_lo)):
        Me, Mo = s * A[0::2], s * A[1::2]                             # [128, 256]
        out[f'L1{x}_w1r'] = hs([Me, -Mo])
        out[f'L1{x}_w2r'] = hs([Me, Mo])
        out[f'L1{x}_w1i'] = hs([Mo, Me])
        out[f'L1{x}_w2i'] = hs([-Mo, Me])
    out['Be1_lo'], out['Bo1_lo'] = A_lo[0::2], A_lo[1::2]
    out['Be1_hi'], out['Bo1_hi'] = A_hi[0::2], A_hi[1::2]
    # block-diagonal row-stage variants: one wide matmul covers all the
    # images packed along the partition (K) dim of the quad/pair tiles.
    out['M3_bd2'] = np.kron(np.eye(2), out['M3_lo'])
    for nm in ('Be3_lo', 'Bo3_lo', 'Be3_hi', 'Bo3_hi'):
        out[nm + '_bd4'] = np.kron(np.eye(4), out[nm])
        del out[nm]
    for nm in ('Be2_lo', 'Bo2_lo', 'Be2_hi', 'Bo2_hi'):
        out[nm + '_bd2'] = np.kron(np.eye(2), out[nm])
        del out[nm]
    return {k: np.ascontiguousarray(v, np.float16) for k, v in out.items()}


MAT_SHAPES = {
    'M3_lo': (64, 128),
    'L3_hi_R': (64, 256), 'L3_hi_I': (64, 256),
    'L3_q_R': (128, 256), 'L3_q_I': (128, 256),
    'M3_bd2': (128, 256),
    'Be3_lo_bd4': (128, 512), 'Bo3_lo_bd4': (128, 512),
    'Be3_hi_bd4': (128, 512), 'Bo3_hi_bd4': (128, 512),
    'M2_lo': (128, 256),
    'L2_hi_R': (128, 512), 'L2_hi_I': (128, 512),
    'L2_lo_R': (128, 512), 'L2_lo_I': (128, 512),
    'Be2_lo_bd2': (128, 512), 'Bo2_lo_bd2': (128, 512),
    'Be2_hi_bd2': (128, 512), 'Bo2_hi_bd2': (128, 512),
    'Alo_a': (128, 256), 'Alo_b': (128, 256),
    'L1hi_w1r': (128, 512), 'L1hi_w2r': (128, 512),
    'L1hi_w1i': (128, 512), 'L1hi_w2i': (128, 512),
    'L1lo_w1r': (128, 512), 'L1lo_w2r': (128, 512),
    'L1lo_w1i': (128, 512), 'L1lo_w2i': (128, 512),
    'Be1_lo': (128, 256), 'Bo1_lo': (128, 256),
    'Be1_hi': (128, 256), 'Bo1_hi': (128, 256),
}

BLOB_A0 = ['M3_lo', 'L3_hi_R', 'L3_hi_I', 'L3_q_R', 'L3_q_I']
BLOB_A0R = ['M3_bd2', 'Be3_lo_bd4', 'Bo3_lo_bd4',
            'Be3_hi_bd4', 'Bo3_hi_bd4']
BLOB_A1 = ['M2_lo', 'L2_hi_R', 'L2_hi_I', 'L2_lo_R', 'L2_lo_I',
           'Be2_lo_bd2', 'Bo2_lo_bd2', 'Be2_hi_bd2', 'Bo2_hi_bd2']
BLOB_B = ['Alo_a', 'Alo_b',
          'L1hi_w1r', 'L1hi_w2r', 'L1hi_w1i', 'L1hi_w2i',
          'L1lo_w1r', 'L1lo_w2r', 'L1lo_w1i', 'L1lo_w2i',
          'Be1_lo', 'Bo1_lo', 'Be1_hi', 'Bo1_hi']
BLOB_A0_COLS = sum(MAT_SHAPES[n][1] for n in BLOB_A0)
BLOB_A0R_COLS = sum(MAT_SHAPES[n][1] for n in BLOB_A0R)
BLOB_A1_COLS = sum(MAT_SHAPES[n][1] for n in BLOB_A1)
BLOB_B_COLS = sum(MAT_SHAPES[n][1] for n in BLOB_B)


def pack_blobs(mats):
    def pack(names, cols):
        blob = np.zeros((128, cols), np.float16)
        c = 0
        for n in names:
            K, N = MAT_SHAPES[n]
            for r in range(128 // K):  # replicate K<128 mats across parts
                blob[r * K:(r + 1) * K, c:c + N] = mats[n]
            c += N
        return blob
    return (pack(BLOB_A0, BLOB_A0_COLS), pack(BLOB_A0R, BLOB_A0R_COLS),
            pack(BLOB_A1, BLOB_A1_COLS), pack(BLOB_B, BLOB_B_COLS))


# ---------------------------------------------------------------------------
# Bass kernel
# ---------------------------------------------------------------------------
def split_excess_waits(nc, max_waits=1):
    """walrus CTRL codegen allows only one sem wait per instruction; move
    excess waits onto NoOps inserted just before the offending instruction."""
    ctr = 0
    for fn in nc.m.functions:
        for bb in fn.blocks:
            new_list = []
            for inst in bb.instructions:
                si = inst.sync_info
                if si is not None and si.on_wait and len(si.on_wait) > max_waits:
                    waits = list(si.on_wait)
                    keep, extra = waits[:max_waits], waits[max_waits:]
                    for i in range(0, len(extra), max_waits):
                        nop = mybir.InstNoOp(
                            name=f"wait_split_{ctr}", ins=[], outs=[])
                        ctr += 1
                        nop.engine = inst.engine
                        nop.sync_info = mybir.SyncInfo(
                            on_wait=extra[i:i + max_waits], on_update=[])
                        nc.register_instruction(nop)
                        new_list.append(nop)
                    inst.sync_info = mybir.SyncInfo(
                        on_wait=keep,
                        on_update=list(si.on_update) if si.on_update else [])
                new_list.append(inst)
            bb.instructions[:] = new_list
    return ctr


def build_nc():
    nc = bass.Bass()
    z3b_d = nc.dram_tensor("z3b", [64, 1024], F16, kind="ExternalInput")
    lh3b_d = nc.dram_tensor("lh3b", [64, 1024], F16, kind="ExternalInput")
    q3b_d = nc.dram_tensor("q3b", [128, 1024], F16, kind="ExternalInput")
    lh2b_d = nc.dram_tensor("lh2b", [128, 2048], F16, kind="ExternalInput")
    hl2b_d = nc.dram_tensor("hl2b", [128, 2048], F16, kind="ExternalInput")
    hh2b_d = nc.dram_tensor("hh2b", [128, 2048], F16, kind="ExternalInput")
    yh0b_d = [nc.dram_tensor(f"yh0b{g}", [128, 6144], F16,
                             kind="ExternalInput") for g in range(4)]
    out_d = nc.dram_tensor("out", [IMGS_PER_CORE, 256, 256], F16,
                           kind="ExternalOutput")
    matsA0_d = nc.dram_tensor("matsA0", [128, BLOB_A0_COLS], F16,
                              kind="ExternalInput")
    matsA0R_d = nc.dram_tensor("matsA0R", [128, BLOB_A0R_COLS], F16,
                               kind="ExternalInput")
    matsA1_d = nc.dram_tensor("matsA1", [128, BLOB_A1_COLS], F16,
                              kind="ExternalInput")
    matsB_d = nc.dram_tensor("matsB", [128, BLOB_B_COLS], F16,
                             kind="ExternalInput")

    with TileContext(nc) as tc:
        with tc.tile_pool(name="mats", bufs=1) as matpool, \
             tc.tile_pool(name="ins", bufs=1) as inpool, \
             tc.tile_pool(name="zs", bufs=1) as zpool, \
             tc.tile_pool(name="mid", bufs=2) as midpool, \
             tc.tile_pool(name="outp", bufs=3) as outpool, \
             tc.tile_pool(name="psbig", bufs=2, space="PSUM") as psbig, \
             tc.tile_pool(name="pssml", bufs=4, space="PSUM") as pssml:

            # --- matrix blobs: separate tiles so deps stay fine-grained;
            # the tiny L3-col blob lands first so img0 starts early ---
            blobA0_t = matpool.tile([128, BLOB_A0_COLS], F16, tag="blobA0")
            nc.scalar.dma_start(out=blobA0_t[:], in_=matsA0_d[:])
            blobA0R_t = matpool.tile([128, BLOB_A0R_COLS], F16,
                                     tag="blobA0R")
            nc.scalar.dma_start(out=blobA0R_t[:], in_=matsA0R_d[:])
            blobA1_t = matpool.tile([128, BLOB_A1_COLS], F16, tag="blobA1")
            nc.scalar.dma_start(out=blobA1_t[:], in_=matsA1_d[:])
            blobB_t = matpool.tile([128, BLOB_B_COLS], F16, tag="blobB")
            nc.scalar.dma_start(out=blobB_t[:], in_=matsB_d[:])
            mats = {}
            mat_loc = {}
            for blob_t, names in ((blobA0_t, BLOB_A0), (blobA0R_t, BLOB_A0R),
                                  (blobA1_t, BLOB_A1), (blobB_t, BLOB_B)):
                c = 0
                for n in names:
                    K, N = MAT_SHAPES[n]
                    mats[n] = blob_t[0:K, c:c + N]
                    mat_loc[n] = (blob_t, c)
                    c += N

            def mat_at(name, poff):
                blob, c = mat_loc[name]
                K, N = MAT_SHAPES[name]
                return blob[poff:poff + K, c:c + N]

            # --- batched input sweeps: host pre-packed blobs, one
            # contiguous DMA each ---
            z3all = {}
            lh3all = {}
            q3all = {}
            for h in range(2):
                cs = slice(h * 512, (h + 1) * 512)
                z3all[h] = inpool.tile([64, 512], F16, tag=f"z3all{h}",
                                       name=f"z3all{h}")
                nc.gpsimd.dma_start(out=z3all[h][:], in_=z3b_d[:, cs])
                lh3all[h] = inpool.tile([64, 512], F16, tag=f"lh3all{h}",
                                        name=f"lh3all{h}")
                nc.gpsimd.dma_start(out=lh3all[h][:], in_=lh3b_d[:, cs])
                q3all[h] = inpool.tile([128, 512], F16, tag=f"q3all{h}",
                                       name=f"q3all{h}")
                nc.gpsimd.dma_start(out=q3all[h][:], in_=q3b_d[:, cs])
            lh2all = inpool.tile([128, 16 * 128], F16, tag="lh2all")
            nc.gpsimd.dma_start(out=lh2all[:], in_=lh2b_d[:])
            hl2all = inpool.tile([128, 16 * 128], F16, tag="hl2all")
            nc.gpsimd.dma_start(out=hl2all[:], in_=hl2b_d[:])
            hh2all = inpool.tile([128, 16 * 128], F16, tag="hh2all")
            nc.gpsimd.dma_start(out=hh2all[:], in_=hh2b_d[:])
            yh0g = {}
            for g in range(4):
                t = inpool.tile([128, 4 * 1536], F16, tag=f"yh0g{g}",
                                name=f"yh0g{g}")
                nc.sync.dma_start(out=t[:], in_=yh0b_d[g][:])
                yh0g[g] = t

            z2q = {q: zpool.tile([128, 512], F16, tag=f"z2q_{q}",
                                 name=f"z2q_{q}")
                   for q in range(IMGS_PER_CORE // 4)}
            z1q = {q: zpool.tile([128, 2048], F16, tag=f"z1q_{q}",
                                 name=f"z1q_{q}")
                   for q in range(IMGS_PER_CORE // 4)}

            def z2s_ap(img):
                return z2q[img // 4][:, (img % 4) * 128:(img % 4) * 128 + 128]

            def z1s_ap(img):
                return z1q[img // 4][:, (img % 4) * 512:(img % 4) * 512 + 512]

            def mm(out_ap, lhsT, rhs_name, start, stop, poff=0):
                rhs = mats[rhs_name] if poff == 0 else mat_at(rhs_name, poff)
                nc.tensor.matmul(out_ap, lhsT, rhs, start=start, stop=stop)

            # ===========================================================
            # Phase L3: quad-packed col stages; emission is software-
            # pipelined (col of quad g+1 precedes rows of quad g) so the
            # in-order PE queue never heads-of-line-blocks on copies
            # ===========================================================
            if True:
                l3t = {}

                def l3_col(g):
                    c0 = (g % 2) * 256
                    zt, lt, qt = z3all[g // 2], lh3all[g // 2], q3all[g // 2]
                    p3 = psbig.tile([128, 1024], F32, tag="p3",
                                       name=f"p3_{g}")
                    mm(p3[:, 0:128], zt[:, c0:c0 + 128], 'M3_lo',
                       True, True)
                    mm(p3[:, 128:256], zt[:, c0 + 128:c0 + 256], 'M3_lo',
                       True, True)
                    lq = lt[:, c0:c0 + 256]
                    mm(p3[:, 256:512], lq[:, 0::2], 'L3_hi_R', True, False)
                    mm(p3[:, 256:512], lq[:, 1::2], 'L3_hi_I', False, True)
                    qq = qt[:, c0:c0 + 256]
                    mm(p3[:, 512:768], qq[:, 0::2], 'L3_q_R', True, False)
                    mm(p3[:, 512:768], qq[:, 1::2], 'L3_q_I', False, True)
                    y1z_s = midpool.tile([128, 256], F16, tag="y1z3",
                                         name=f"y1z3_{g}")
                    nc.scalar.copy(y1z_s[:], p3[:, 0:256])
                    y1b_s = midpool.tile([128, 256], F16, tag="y1b3",
                                         name=f"y1b3_{g}")
                    nc.vector.tensor_copy(out=y1b_s[:], in_=p3[:, 256:512])
                    y2b_s = midpool.tile([128, 256], F16, tag="y2b3",
                                         name=f"y2b3_{g}")
                    nc.vector.tensor_copy(out=y2b_s[:], in_=p3[:, 512:768])
                    l3t[g] = (y1z_s, y1b_s, y2b_s)

                def l3_row(g):
                    y1z_s, y1b_s, y2b_s = l3t.pop(g)
                    prow = pssml.tile([128, 512], F32, tag="ps",
                                         name=f"p3r_{g}")
                    # imgs live at N cols 128*i via block-diagonal rhs;
                    # the full-width band mm starts the accumulation, the
                    # narrow lowpass mms accumulate into sub-ranges after
                    mm(prow[:], y1b_s[:, 0:128], 'Be3_lo_bd4', True, False)
                    mm(prow[:], y1b_s[:, 128:256], 'Bo3_lo_bd4',
                       False, False)
                    mm(prow[:], y2b_s[:, 0:128], 'Be3_hi_bd4', False, False)
                    mm(prow[:], y2b_s[:, 128:256], 'Bo3_hi_bd4',
                       False, False)
                    mm(prow[:, 0:256], y1z_s[:, 0:128], 'M3_bd2',
                       False, True)
                    mm(prow[:, 256:512], y1z_s[:, 128:256], 'M3_bd2',
                       False, True)
                    for i in range(4):
                        img = 4 * g + i
                        zp = prow[:, i * 128:(i + 1) * 128]
                        if i % 2 == 0:
                            nc.scalar.copy(z2s_ap(img), zp)
                        else:
                            nc.vector.tensor_copy(out=z2s_ap(img), in_=zp)

                l3_col(0)
                for g in range(1, 4):
                    l3_col(g)
                    l3_row(g - 1)
                l3_row(3)

            # ===========================================================
            # Phase L2: pair-packed col stages, software-pipelined with
            # per-img row stages (lhsT partition offsets)
            # ===========================================================
            if True:
                l2t = {}

                def l2_col(p):
                    cc = p * 256
                    pA = pssml.tile([128, 512], F32, tag="ps",
                                       name=f"p2A_{p}")
                    mm(pA[:, 0:256], z2s_ap(2 * p), 'M2_lo', True, True)
                    mm(pA[:, 256:512], z2s_ap(2 * p + 1), 'M2_lo',
                       True, True)
                    pB = pssml.tile([128, 512], F32, tag="ps",
                                       name=f"p2B_{p}")
                    lp = lh2all[:, cc:cc + 256]
                    mm(pB[:], lp[:, 0::2], 'L2_hi_R', True, False)
                    mm(pB[:], lp[:, 1::2], 'L2_hi_I', False, True)
                    pC = pssml.tile([128, 512], F32, tag="ps",
                                       name=f"p2C_{p}")
                    hp = hl2all[:, cc:cc + 256]
                    hq = hh2all[:, cc:cc + 256]
                    mm(pC[:], hp[:, 0::2], 'L2_lo_R', True, False)
                    mm(pC[:], hp[:, 1::2], 'L2_lo_I', False, False)
                    mm(pC[:], hq[:, 0::2], 'L2_hi_R', False, False)
                    mm(pC[:], hq[:, 1::2], 'L2_hi_I', False, True)
                    y1zT_s = midpool.tile([128, 512], F16, tag="y1zT2",
                                          name=f"y1zT2_{p}")
                    nc.scalar.copy(y1zT_s[:], pA[:])
                    b1_s = midpool.tile([128, 512], F16, tag="b1_2",
                                        name=f"b1_2_{p}")
                    nc.vector.tensor_copy(out=b1_s[:], in_=pB[:])
                    b2_s = midpool.tile([128, 512], F16, tag="b2_2",
                                        name=f"b2_2_{p}")
                    nc.vector.tensor_copy(out=b2_s[:], in_=pC[:])
                    l2t[p] = (y1zT_s, b1_s, b2_s)

                def l2_row(p):
                    y1zT_s, b1_s, b2_s = l2t.pop(p)
                    a, b = 2 * p, 2 * p + 1
                    for m in range(2):
                        # chunk m of both imgs: N = [a w_out 256 | b w_out
                        # 256], bands via block-diagonal rhs over the pair
                        p2r = pssml.tile([128, 512], F32, tag="ps",
                                            name=f"p2r_{p}_{m}")
                        msl = slice(m * 128, (m + 1) * 128)
                        osl = slice(256 + m * 128, 256 + (m + 1) * 128)
                        mm(p2r[:], b1_s[:, msl], 'Be2_lo_bd2', True, False)
                        mm(p2r[:], b1_s[:, osl], 'Bo2_lo_bd2', False, False)
                        mm(p2r[:], b2_s[:, msl], 'Be2_hi_bd2', False, False)
                        mm(p2r[:], b2_s[:, osl], 'Bo2_hi_bd2', False, False)
                        mm(p2r[:, 0:256], y1zT_s[:, m * 128:(m + 1) * 128],
                           'M2_lo', False, True)
                        mm(p2r[:, 256:512],
                           y1zT_s[:, 256 + m * 128:256 + (m + 1) * 128],
                           'M2_lo', False, True)
                        za, zb = z1s_ap(a), z1s_ap(b)
                        if m == 0:
                            nc.scalar.copy(za[:, 0:256], p2r[:, 0:256])
                            nc.vector.tensor_copy(out=zb[:, 0:256],
                                                  in_=p2r[:, 256:512])
                        else:
                            nc.scalar.copy(za[:, 256:512], p2r[:, 0:256])
                            nc.vector.tensor_copy(out=zb[:, 256:512],
                                                  in_=p2r[:, 256:512])

                l2_col(0)
                for p in range(1, 8):
                    l2_col(p)
                    l2_row(p - 1)
                l2_row(7)

            # ===========================================================
            # Phase L1: z1 + yh0 bands -> out, software-pipelined
            # ===========================================================
            if True:
                l1t = {}

                def l1_col(img):
                    yh0t = yh0g[img // 4]
                    ib = (img % 4) * 1536
                    o_t = {o: yh0t[:, ib + o * 256:ib + (o + 1) * 256]
                           for o in range(6)}
                    z1_s = z1s_ap(img)
                    # phase A: y1 = band + lowpass, merged in w-polyphase
                    # layout [E(h 256) | O(h 256)]  (partitions = w')
                    p1a = pssml.tile([128, 512], F32, tag="ps",
                                        name=f"p1a_{img}")
                    y1_p = p1a[:]
                    mm(y1_p, o_t[0][:, 0::2], 'L1hi_w1r', True, False)
                    mm(y1_p, o_t[5][:, 0::2], 'L1hi_w2r', False, False)
                    mm(y1_p, o_t[0][:, 1::2], 'L1hi_w1i', False, False)
                    mm(y1_p, o_t[5][:, 1::2], 'L1hi_w2i', False, False)
                    mm(p1a[:, 0:256], z1_s[:, 0:256:2], 'Alo_a',
                       False, False)
                    mm(p1a[:, 0:256], z1_s[:, 256:512:2], 'Alo_b',
                       False, True)
                    mm(p1a[:, 256:512], z1_s[:, 1:256:2], 'Alo_a',
                       False, False)
                    mm(p1a[:, 256:512], z1_s[:, 257:512:2], 'Alo_b',
                       False, True)
                    y1_s = midpool.tile([128, 512], F16, tag="y1m",
                                        name=f"y1m_{img}")
                    nc.vector.tensor_copy(out=y1_s[:], in_=y1_p)

                    # phase B: y2b e|o [0:512)
                    p1b = pssml.tile([128, 512], F32, tag="ps",
                                        name=f"p1b_{img}")
                    y2b_p = p1b[:]
                    mm(y2b_p, o_t[2][:, 0::2], 'L1lo_w1r', True, False)
                    mm(y2b_p, o_t[3][:, 0::2], 'L1lo_w2r', False, False)
                    mm(y2b_p, o_t[2][:, 1::2], 'L1lo_w1i', False, False)
                    mm(y2b_p, o_t[3][:, 1::2], 'L1lo_w2i', False, False)
                    mm(y2b_p, o_t[1][:, 0::2], 'L1hi_w1r', False, False)
                    mm(y2b_p, o_t[4][:, 0::2], 'L1hi_w2r', False, False)
                    mm(y2b_p, o_t[1][:, 1::2], 'L1hi_w1i', False, False)
                    mm(y2b_p, o_t[4][:, 1::2], 'L1hi_w2i', False, True)
                    y2b1_s = midpool.tile([128, 512], F16, tag="y2b1",
                                          name=f"y2b1_{img}")
                    nc.vector.tensor_copy(out=y2b1_s[:], in_=y2b_p)
                    l1t[img] = (y1_s, y2b1_s)

                def l1_row(img):
                    y1_s, y2b1_s = l1t.pop(img)
                    # row stage -> out [256, 256] in two h-chunks; single
                    # store DMA per image ([a p] x <- p [a x])
                    p1r = pssml.tile([128, 512], F32, tag="ps",
                                        name=f"p1r_{img}")
                    ot = outpool.tile([128, 512], F16, tag="ot",
                                      name=f"ot_{img}")
                    for m in range(2):
                        oc = p1r[:, m * 256:(m + 1) * 256]
                        msl = slice(m * 128, (m + 1) * 128)
                        osl = slice(256 + m * 128, 256 + (m + 1) * 128)
                        mm(oc, y1_s[:, msl], 'Be1_lo', True, False)
                        mm(oc, y1_s[:, osl], 'Bo1_lo', False, False)
                        mm(oc, y2b1_s[:, msl], 'Be1_hi', False, False)
                        mm(oc, y2b1_s[:, osl], 'Bo1_hi', False, True)
                        if m == 0:
                            nc.scalar.copy(ot[:, 0:256], oc)
                        else:
                            nc.vector.tensor_copy(out=ot[:, 256:512], in_=oc)
                    nc.gpsimd.dma_start(
                        out=out_d[img].rearrange("(a p) x -> p a x", a=2),
                        in_=ot.rearrange("p (a x) -> p a x", a=2))

                l1_col(0)
                for img in range(1, IMGS_PER_CORE):
                    l1_col(img)
                    l1_row(img - 1)
                l1_row(IMGS_PER_CORE - 1)

    split_excess_waits(nc)
    return nc


# ---------------------------------------------------------------------------
# Entry point
# ---------------------------------------------------------------------------
_NC_CACHE = []
_LAST_RESULT = []  # last BassKernelResults (exec_time_ns when BASS_TRACE=1)


def _axon_reset():
    try:
        import ctypes
        lib = ctypes.CDLL('/opt/axon/libaxon_pjrt.so')
        lib.axon_reset.restype = ctypes.c_int64
        lib.axon_reset()
    except Exception:
        pass


def _pack_pairs(yh, o_pair):
    """[16, 6, n, n, 2] -> [2n, 16*2n]: orientation pair partition-stacked,
    image-major columns with (w, ri) interleave."""
    a = yh[:, o_pair]                       # [16, 2, n, n, 2]
    n = a.shape[2]
    return np.ascontiguousarray(
        a.transpose(1, 2, 0, 3, 4).reshape(2 * n, 16 * 2 * n))


def kernel(yl, yh0, yh1, yh2, g0o, g1o, g0a, g0b, g1a, g1b):
    yl = np.asarray(yl, np.float16)
    yh0 = np.asarray(yh0, np.float16)
    yh1 = np.asarray(yh1, np.float16)
    yh2 = np.asarray(yh2, np.float16)
    assert yl.shape == (8, 16, 64, 64)

    mats = build_matrices(g0o, g1o, g0a, g0b, g1a, g1b)
    blobA0, blobA0R, blobA1, blobB = pack_blobs(mats)
    if not _NC_CACHE:
        _NC_CACHE.append(build_nc())
    nc = _NC_CACHE[0]

    in_maps = []
    for core in range(N_CORES):
        m = {"matsA0": blobA0, "matsA0R": blobA0R,
             "matsA1": blobA1, "matsB": blobB}
        m["z3b"] = np.ascontiguousarray(
            yl[core].transpose(1, 0, 2).reshape(64, 1024))
        m["lh3b"] = _pack_pairs(yh2[core], [0, 5])
        m["q3b"] = np.vstack([_pack_pairs(yh2[core], [2, 3]),
                              _pack_pairs(yh2[core], [1, 4])])
        m["lh2b"] = _pack_pairs(yh1[core], [0, 5])
        m["hl2b"] = _pack_pairs(yh1[core], [2, 3])
        m["hh2b"] = _pack_pairs(yh1[core], [1, 4])
        # yh0: [16, 6, 128, 256] -> [128, 16*1536] o-major per img, in
        # 4 groups of 4 imgs
        y0 = yh0[core].reshape(16, 6, 128, 256)
        y0 = y0.transpose(2, 0, 1, 3)       # [128, 16, 6, 256]
        for g in range(4):
            m[f"yh0b{g}"] = np.ascontiguousarray(
                y0[:, 4 * g:4 * g + 4].reshape(128, 4 * 1536))
        in_maps.append(m)

    try:
        res = run_bass_kernel_spmd(nc, in_maps, list(range(N_CORES)))
    except Exception as e:  # wedged exec unit: reset the axon device, retry
        if "UNAVAILABLE" not in str(e) and "unrecoverable" not in str(e):
            raise
        _axon_reset()
        res = run_bass_kernel_spmd(nc, in_maps, list(range(N_CORES)))
    _LAST_RESULT.clear()
    _LAST_RESULT.append(res)
    out = np.stack([res.results[i]["out"] for i in range(N_CORES)], axis=0)
    return np.ascontiguousarray(out.astype(np.float32))


# revision 37
# speedup vs baseline: 1.1603x; 1.1603x over previous
"""DTCWT 3-level inverse on 8 Trainium2 NeuronCores.

Every filtering stage is a banded matmul on the tensor engine in fp16
(PSUM accumulates fp32; ~7e-4 total rel err vs the 2e-2 gate).

All stages use "data as lhsT" mode: matmul(out, lhsT=data[K=h, M=w],
rhs=mat[K=h, N=h_out]) contracts over the partition dim of the data and
yields the filtered image TRANSPOSED ([w, h_out]); column and row stages
then alternate orientation naturally with zero explicit transposes.

The c2q band construction is folded into the matrices; at L1 the lowpass
path is additionally merged into the band polyphase layout ([E|O] w-planes)
so the final row stage is 4 accumulation passes instead of 6.

Schedule: phase-major (L3 x16, L2 x16, L1 x16) with double/triple-buffered
PSUM pools so neighbouring images' matmuls hide each other's copy latency.
DMA queue slots cost ~600ns regardless of size, so all loads are batched
into a handful of giant multi-dim DMAs (2 matrix blobs, 9 input sweeps,
1 store per image).

Sharding: pure data parallel over batch N (8 cores x 16 channels each).
"""
import sys

for _p in ('/opt/trn_rl_repo',):
    if _p not in sys.path:
        sys.path.append(_p)

import numpy as np
import concourse.bass as bass
import concourse.mybir as mybir
from concourse.tile import TileContext
from concourse.bass_utils import run_bass_kernel_spmd

SQRT_HALF = 0.7071067811865476
N_CORES = 8
IMGS_PER_CORE = 16
F32 = mybir.dt.float32
F16 = mybir.dt.float16


# ---------------------------------------------------------------------------
# Host-side matrix construction (numpy, float64)
# ---------------------------------------------------------------------------
def _conv_rows_valid(x, h):
    hr = h[::-1]
    taps = h.shape[0]
    n = x.shape[-2] - taps + 1
    out = hr[0] * x[..., 0:n, :]
    for k in range(1, taps):
        out = out + hr[k] * x[..., k:k + n, :]
    return out


def _pad_rows_symmetric(x, m):
    pad = [(0, 0)] * (x.ndim - 2) + [(m, m), (0, 0)]
    return np.pad(x, pad, mode='symmetric')


def _colfilter(x, h):
    return _conv_rows_valid(_pad_rows_symmetric(x, h.shape[0] // 2), h)


def _colifilt(x, ha, hb, highpass):
    m = ha.shape[0]
    m2 = m // 2
    r = x.shape[-2]
    xp = _pad_rows_symmetric(x, m2)
    xe = xp[..., 1:r + m - 2:2, :]
    xo = xp[..., 2:r + m - 1:2, :]
    xa, xb = (xe, xo) if highpass else (xo, xe)
    hao, hae = ha[0::2], ha[1::2]
    hbo, hbe = hb[0::2], hb[1::2]
    y0 = _conv_rows_valid(xb, hao)
    y1 = _conv_rows_valid(xa, hbo)
    y2 = _conv_rows_valid(xb, hae)
    y3 = _conv_rows_valid(xa, hbe)
    y = np.stack([y0, y1, y2, y3], axis=-2)
    return y.reshape(y.shape[:-3] + (2 * r, y.shape[-1]))


def _op_matrix(op, n):
    """M[h_in, h_out] with out[h_out, w] = sum_h M[h, h_out] x[h, w]."""
    return np.ascontiguousarray(op(np.eye(n, dtype=np.float64)).T)


def build_matrices(g0o, g1o, g0a, g0b, g1a, g1b):
    """All device matrices as {name: fp16 ndarray}."""
    g0o = np.asarray(g0o, np.float64)
    g1o = np.asarray(g1o, np.float64)
    g0a = np.asarray(g0a, np.float64)
    g0b = np.asarray(g0b, np.float64)
    g1a = np.asarray(g1a, np.float64)
    g1b = np.asarray(g1b, np.float64)
    s = SQRT_HALF
    hs, vs = np.hstack, np.vstack
    out = {}

    def upsample_level(R, tag):
        Mlo = _op_matrix(lambda x: _colifilt(x, g0b, g0a, False), R)  # [R, 2R]
        Mhi = _op_matrix(lambda x: _colifilt(x, g1b, g1a, True), R)
        Me_h, Mo_h = s * Mhi[0::2], s * Mhi[1::2]                     # [R/2, 2R]
        Me_l, Mo_l = s * Mlo[0::2], s * Mlo[1::2]
        out[f'M{tag}_lo'] = Mlo
        # pair-stacked [w1; w2] col rhs, e|o column-concatenated
        #   e: w1r*Me + w2r*Me + w1i*Mo - w2i*Mo
        #   o: -w1r*Mo + w2r*Mo + w1i*Me + w2i*Me
        out[f'L{tag}_hi_R'] = hs([vs([Me_h, Me_h]), vs([-Mo_h, Mo_h])])
        out[f'L{tag}_hi_I'] = hs([vs([Mo_h, -Mo_h]), vs([Me_h, Me_h])])
        out[f'L{tag}_lo_R'] = hs([vs([Me_l, Me_l]), vs([-Mo_l, Mo_l])])
        out[f'L{tag}_lo_I'] = hs([vs([Mo_l, -Mo_l]), vs([Me_l, Me_l])])
        # row stage (polyphase-column recombination)
        out[f'Be{tag}_lo'], out[f'Bo{tag}_lo'] = Mlo[0::2], Mlo[1::2]
        out[f'Be{tag}_hi'], out[f'Bo{tag}_hi'] = Mhi[0::2], Mhi[1::2]

    upsample_level(64, '3')
    upsample_level(128, '2')
    # L3 quad stacks: [hl pair (lo mats); hh pair (hi mats)], K=128
    out['L3_q_R'] = vs([out['L3_lo_R'], out['L3_hi_R']])
    out['L3_q_I'] = vs([out['L3_lo_I'], out['L3_hi_I']])
    del out['L3_lo_R'], out['L3_lo_I']  # only used inside the quad at L3

    # L1 (colfilter, size-preserving, n=256)
    A_lo = _op_matrix(lambda x: _colfilter(x, g0o), 256)              # [256, 256]
    A_hi = _op_matrix(lambda x: _colfilter(x, g1o), 256)
    out['Alo_a'], out['Alo_b'] = A_lo[0:128], A_lo[128:256]
    for x, A in (('hi', A_hi), ('lo', A_lo)):
        Me, Mo = s * A[0::2], s * A[1::2]                             # [128, 256]
        out[f'L1{x}_w1r'] = hs([Me, -Mo])
        out[f'L1{x}_w2r'] = hs([Me, Mo])
        out[f'L1{x}_w1i'] = hs([Mo, Me])
        out[f'L1{x}_w2i'] = hs([-Mo, Me])
    out['Be1_lo'], out['Bo1_lo'] = A_lo[0::2], A_lo[1::2]
    out['Be1_hi'], out['Bo1_hi'] = A_hi[0::2], A_hi[1::2]
    # block-diagonal row-stage variants: one wide matmul covers all the
    # images packed along the partition (K) dim of the quad/pair tiles.
    out['M3_bd2'] = np.kron(np.eye(2), out['M3_lo'])
    for nm in ('Be3_lo', 'Bo3_lo', 'Be3_hi', 'Bo3_hi'):
        out[nm + '_bd4'] = np.kron(np.eye(4), out[nm])
        del out[nm]
    for nm in ('Be2_lo', 'Bo2_lo', 'Be2_hi', 'Bo2_hi'):
        out[nm + '_bd2'] = np.kron(np.eye(2), out[nm])
        del out[nm]
    return {k: np.ascontiguousarray(v, np.float16) for k, v in out.items()}


MAT_SHAPES = {
    'M3_lo': (64, 128),
    'L3_hi_R': (64, 256), 'L3_hi_I': (64, 256),
    'L3_q_R': (128, 256), 'L3_q_I': (128, 256),
    'M3_bd2': (128, 256),
    'Be3_lo_bd4': (128, 512), 'Bo3_lo_bd4': (128, 512),
    'Be3_hi_bd4': (128, 512), 'Bo3_hi_bd4': (128, 512),
    'M2_lo': (128, 256),
    'L2_hi_R': (128, 512), 'L2_hi_I': (128, 512),
    'L2_lo_R': (128, 512), 'L2_lo_I': (128, 512),
    'Be2_lo_bd2': (128, 512), 'Bo2_lo_bd2': (128, 512),
    'Be2_hi_bd2': (128, 512), 'Bo2_hi_bd2': (128, 512),
    'Alo_a': (128, 256), 'Alo_b': (128, 256),
    'L1hi_w1r': (128, 512), 'L1hi_w2r': (128, 512),
    'L1hi_w1i': (128, 512), 'L1hi_w2i': (128, 512),
    'L1lo_w1r': (128, 512), 'L1lo_w2r': (128, 512),
    'L1lo_w1i': (128, 512), 'L1lo_w2i': (128, 512),
    'Be1_lo': (128, 256), 'Bo1_lo': (128, 256),
    'Be1_hi': (128, 256), 'Bo1_hi': (128, 256),
}

BLOB_A0 = ['M3_lo', 'L3_hi_R', 'L3_hi_I', 'L3_q_R', 'L3_q_I']
BLOB_A0R = ['M3_bd2', 'Be3_lo_bd4', 'Bo3_lo_bd4',
            'Be3_hi_bd4', 'Bo3_hi_bd4']
BLOB_A1 = ['M2_lo', 'L2_hi_R', 'L2_hi_I', 'L2_lo_R', 'L2_lo_I',
           'Be2_lo_bd2', 'Bo2_lo_bd2', 'Be2_hi_bd2', 'Bo2_hi_bd2']
BLOB_B = ['Alo_a', 'Alo_b',
          'L1hi_w1r', 'L1hi_w2r', 'L1hi_w1i', 'L1hi_w2i',
          'L1lo_w1r', 'L1lo_w2r', 'L1lo_w1i', 'L1lo_w2i',
          'Be1_lo', 'Bo1_lo', 'Be1_hi', 'Bo1_hi']
BLOB_A0_COLS = sum(MAT_SHAPES[n][1] for n in BLOB_A0)
BLOB_A0R_COLS = sum(MAT_SHAPES[n][1] for n in BLOB_A0R)
BLOB_A1_COLS = sum(MAT_SHAPES[n][1] for n in BLOB_A1)
BLOB_B_COLS = sum(MAT_SHAPES[n][1] for n in BLOB_B)


def pack_blobs(mats):
    def pack(names, cols):
        blob = np.zeros((128, cols), np.float16)
        c = 0
        for n in names:
            K, N = MAT_SHAPES[n]
            for r in range(128 // K):  # replicate K<128 mats across parts
                blob[r * K:(r + 1) * K, c:c + N] = mats[n]
            c += N
        return blob
    return (pack(BLOB_A0, BLOB_A0_COLS), pack(BLOB_A0R, BLOB_A0R_COLS),
            pack(BLOB_A1, BLOB_A1_COLS), pack(BLOB_B, BLOB_B_COLS))


# ---------------------------------------------------------------------------
# Bass kernel
# ---------------------------------------------------------------------------
def split_excess_waits(nc, max_waits=1):
    """walrus CTRL codegen allows only one sem wait per instruction; move
    excess waits onto NoOps inserted just before the offending instruction."""
    ctr = 0
    for fn in nc.m.functions:
        for bb in fn.blocks:
            new_list = []
            for inst in bb.instructions:
                si = inst.sync_info
                if si is not None and si.on_wait and len(si.on_wait) > max_waits:
                    waits = list(si.on_wait)
                    keep, extra = waits[:max_waits], waits[max_waits:]
                    for i in range(0, len(extra), max_waits):
                        nop = mybir.InstNoOp(
                            name=f"wait_split_{ctr}", ins=[], outs=[])
                        ctr += 1
                        nop.engine = inst.engine
                        nop.sync_info = mybir.SyncInfo(
                            on_wait=extra[i:i + max_waits], on_update=[])
                        nc.register_instruction(nop)
                        new_list.append(nop)
                    inst.sync_info = mybir.SyncInfo(
                        on_wait=keep,
                        on_update=list(si.on_update) if si.on_update else [])
                new_list.append(inst)
            bb.instructions[:] = new_list
    return ctr


def build_nc():
    nc = bass.Bass()
    z3b_d = nc.dram_tensor("z3b", [64, 1024], F16, kind="ExternalInput")
    lh3b_d = nc.dram_tensor("lh3b", [64, 1024], F16, kind="ExternalInput")
    q3b_d = nc.dram_tensor("q3b", [128, 1024], F16, kind="ExternalInput")
    lh2b_d = nc.dram_tensor("lh2b", [128, 2048], F16, kind="ExternalInput")
    hl2b_d = nc.dram_tensor("hl2b", [128, 2048], F16, kind="ExternalInput")
    hh2b_d = nc.dram_tensor("hh2b", [128, 2048], F16, kind="ExternalInput")
    yh0b_d = [nc.dram_tensor(f"yh0b{g}", [128, 6144], F16,
                             kind="ExternalInput") for g in range(4)]
    out_d = nc.dram_tensor("out", [IMGS_PER_CORE, 256, 256], F16,
                           kind="ExternalOutput")
    matsA0_d = nc.dram_tensor("matsA0", [128, BLOB_A0_COLS], F16,
                              kind="ExternalInput")
    matsA0R_d = nc.dram_tensor("matsA0R", [128, BLOB_A0R_COLS], F16,
                               kind="ExternalInput")
    matsA1_d = nc.dram_tensor("matsA1", [128, BLOB_A1_COLS], F16,
                              kind="ExternalInput")
    matsB_d = nc.dram_tensor("matsB", [128, BLOB_B_COLS], F16,
                             kind="ExternalInput")

    with TileContext(nc) as tc:
        with tc.tile_pool(name="mats", bufs=1) as matpool, \
             tc.tile_pool(name="ins", bufs=1) as inpool, \
             tc.tile_pool(name="zs", bufs=1) as zpool, \
             tc.tile_pool(name="mid", bufs=2) as midpool, \
             tc.tile_pool(name="outp", bufs=3) as outpool, \
             tc.tile_pool(name="psbig", bufs=2, space="PSUM") as psbig, \
             tc.tile_pool(name="pssml", bufs=4, space="PSUM") as pssml:

            # --- matrix blobs: separate tiles so deps stay fine-grained;
            # the tiny L3-col blob lands first so img0 starts early ---
            blobA0_t = matpool.tile([128, BLOB_A0_COLS], F16, tag="blobA0")
            nc.scalar.dma_start(out=blobA0_t[:], in_=matsA0_d[:])
            blobA0R_t = matpool.tile([128, BLOB_A0R_COLS], F16,
                                     tag="blobA0R")
            nc.scalar.dma_start(out=blobA0R_t[:], in_=matsA0R_d[:])
            blobA1_t = matpool.tile([128, BLOB_A1_COLS], F16, tag="blobA1")
            nc.scalar.dma_start(out=blobA1_t[:], in_=matsA1_d[:])
            blobB_t = matpool.tile([128, BLOB_B_COLS], F16, tag="blobB")
            nc.scalar.dma_start(out=blobB_t[:], in_=matsB_d[:])
            mats = {}
            mat_loc = {}
            for blob_t, names in ((blobA0_t, BLOB_A0), (blobA0R_t, BLOB_A0R),
                                  (blobA1_t, BLOB_A1), (blobB_t, BLOB_B)):
                c = 0
                for n in names:
                    K, N = MAT_SHAPES[n]
                    mats[n] = blob_t[0:K, c:c + N]
                    mat_loc[n] = (blob_t, c)
                    c += N

            def mat_at(name, poff):
                blob, c = mat_loc[name]
                K, N = MAT_SHAPES[name]
                return blob[poff:poff + K, c:c + N]

            # --- batched input sweeps: host pre-packed blobs, one
            # contiguous DMA each ---
            z3all = {}
            lh3all = {}
            q3all = {}
            for h in range(2):
                cs = slice(h * 512, (h + 1) * 512)
                z3all[h] = inpool.tile([64, 512], F16, tag=f"z3all{h}",
                                       name=f"z3all{h}")
                nc.gpsimd.dma_start(out=z3all[h][:], in_=z3b_d[:, cs])
                lh3all[h] = inpool.tile([64, 512], F16, tag=f"lh3all{h}",
                                        name=f"lh3all{h}")
                nc.gpsimd.dma_start(out=lh3all[h][:], in_=lh3b_d[:, cs])
                q3all[h] = inpool.tile([128, 512], F16, tag=f"q3all{h}",
                                       name=f"q3all{h}")
                nc.gpsimd.dma_start(out=q3all[h][:], in_=q3b_d[:, cs])
            lh2all = inpool.tile([128, 16 * 128], F16, tag="lh2all")
            nc.gpsimd.dma_start(out=lh2all[:], in_=lh2b_d[:])
            hl2all = inpool.tile([128, 16 * 128], F16, tag="hl2all")
            nc.gpsimd.dma_start(out=hl2all[:], in_=hl2b_d[:])
            hh2all = inpool.tile([128, 16 * 128], F16, tag="hh2all")
            nc.gpsimd.dma_start(out=hh2all[:], in_=hh2b_d[:])
            yh0g = {}
            for g in range(4):
                t = inpool.tile([128, 4 * 1536], F16, tag=f"yh0g{g}",
                                name=f"yh0g{g}")
                nc.gpsimd.dma_start(out=t[:], in_=yh0b_d[g][:])
                yh0g[g] = t

            z2q = {q: zpool.tile([128, 512], F16, tag=f"z2q_{q}",
                                 name=f"z2q_{q}")
                   for q in range(IMGS_PER_CORE // 4)}
            z1q = {q: zpool.tile([128, 2048], F16, tag=f"z1q_{q}",
                                 name=f"z1q_{q}")
                   for q in range(IMGS_PER_CORE // 4)}

            def z2s_ap(img):
                return z2q[img // 4][:, (img % 4) * 128:(img % 4) * 128 + 128]

            def z1s_ap(img):
                return z1q[img // 4][:, (img % 4) * 512:(img % 4) * 512 + 512]

            def mm(out_ap, lhsT, rhs_name, start, stop, poff=0):
                rhs = mats[rhs_name] if poff == 0 else mat_at(rhs_name, poff)
                nc.tensor.matmul(out_ap, lhsT, rhs, start=start, stop=stop)

            # ===========================================================
            # Phase L3: quad-packed col stages; emission is software-
            # pipelined (col of quad g+1 precedes rows of quad g) so the
            # in-order PE queue never heads-of-line-blocks on copies
            # ===========================================================
            if True:
                l3t = {}

                def l3_col(g):
                    c0 = (g % 2) * 256
                    zt, lt, qt = z3all[g // 2], lh3all[g // 2], q3all[g // 2]
                    p3 = psbig.tile([128, 1024], F32, tag="p3",
                                       name=f"p3_{g}")
                    mm(p3[:, 0:128], zt[:, c0:c0 + 128], 'M3_lo',
                       True, True)
                    mm(p3[:, 128:256], zt[:, c0 + 128:c0 + 256], 'M3_lo',
                       True, True)
                    lq = lt[:, c0:c0 + 256]
                    mm(p3[:, 256:512], lq[:, 0::2], 'L3_hi_R', True, False)
                    mm(p3[:, 256:512], lq[:, 1::2], 'L3_hi_I', False, True)
                    qq = qt[:, c0:c0 + 256]
                    mm(p3[:, 512:768], qq[:, 0::2], 'L3_q_R', True, False)
                    mm(p3[:, 512:768], qq[:, 1::2], 'L3_q_I', False, True)
                    y1z_s = midpool.tile([128, 256], F16, tag="y1z3",
                                         name=f"y1z3_{g}")
                    nc.scalar.copy(y1z_s[:], p3[:, 0:256])
                    y1b_s = midpool.tile([128, 256], F16, tag="y1b3",
                                         name=f"y1b3_{g}")
                    nc.vector.tensor_copy(out=y1b_s[:], in_=p3[:, 256:512])
                    y2b_s = midpool.tile([128, 256], F16, tag="y2b3",
                                         name=f"y2b3_{g}")
                    nc.vector.tensor_copy(out=y2b_s[:], in_=p3[:, 512:768])
                    l3t[g] = (y1z_s, y1b_s, y2b_s)

                def l3_row(g):
                    y1z_s, y1b_s, y2b_s = l3t.pop(g)
                    prow = pssml.tile([128, 512], F32, tag="ps",
                                         name=f"p3r_{g}")
                    # imgs live at N cols 128*i via block-diagonal rhs;
                    # the full-width band mm starts the accumulation, the
                    # narrow lowpass mms accumulate into sub-ranges after
                    mm(prow[:], y1b_s[:, 0:128], 'Be3_lo_bd4', True, False)
                    mm(prow[:], y1b_s[:, 128:256], 'Bo3_lo_bd4',
                       False, False)
                    mm(prow[:], y2b_s[:, 0:128], 'Be3_hi_bd4', False, False)
                    mm(prow[:], y2b_s[:, 128:256], 'Bo3_hi_bd4',
                       False, False)
                    mm(prow[:, 0:256], y1z_s[:, 0:128], 'M3_bd2',
                       False, True)
                    mm(prow[:, 256:512], y1z_s[:, 128:256], 'M3_bd2',
                       False, True)
                    for i in range(4):
                        img = 4 * g + i
                        zp = prow[:, i * 128:(i + 1) * 128]
                        if i % 2 == 0:
                            nc.scalar.copy(z2s_ap(img), zp)
                        else:
                            nc.vector.tensor_copy(out=z2s_ap(img), in_=zp)

                l3_col(0)
                for g in range(1, 4):
                    l3_col(g)
                    l3_row(g - 1)
                l3_row(3)

            # ===========================================================
            # Phase L2: pair-packed col stages, software-pipelined with
            # per-img row stages (lhsT partition offsets)
            # ===========================================================
            if True:
                l2t = {}

                def l2_col(p):
                    cc = p * 256
                    pA = pssml.tile([128, 512], F32, tag="ps",
                                       name=f"p2A_{p}")
                    mm(pA[:, 0:256], z2s_ap(2 * p), 'M2_lo', True, True)
                    mm(pA[:, 256:512], z2s_ap(2 * p + 1), 'M2_lo',
                       True, True)
                    pB = pssml.tile([128, 512], F32, tag="ps",
                                       name=f"p2B_{p}")
                    lp = lh2all[:, cc:cc + 256]
                    mm(pB[:], lp[:, 0::2], 'L2_hi_R', True, False)
                    mm(pB[:], lp[:, 1::2], 'L2_hi_I', False, True)
                    pC = pssml.tile([128, 512], F32, tag="ps",
                                       name=f"p2C_{p}")
                    hp = hl2all[:, cc:cc + 256]
                    hq = hh2all[:, cc:cc + 256]
                    mm(pC[:], hp[:, 0::2], 'L2_lo_R', True, False)
                    mm(pC[:], hp[:, 1::2], 'L2_lo_I', False, False)
                    mm(pC[:], hq[:, 0::2], 'L2_hi_R', False, False)
                    mm(pC[:], hq[:, 1::2], 'L2_hi_I', False, True)
                    y1zT_s = midpool.tile([128, 512], F16, tag="y1zT2",
                                          name=f"y1zT2_{p}")
                    nc.scalar.copy(y1zT_s[:], pA[:])
                    b1_s = midpool.tile([128, 512], F16, tag="b1_2",
                                        name=f"b1_2_{p}")
                    nc.vector.tensor_copy(out=b1_s[:], in_=pB[:])
                    b2_s = midpool.tile([128, 512], F16, tag="b2_2",
                                        name=f"b2_2_{p}")
                    nc.vector.tensor_copy(out=b2_s[:], in_=pC[:])
                    l2t[p] = (y1zT_s, b1_s, b2_s)

                def l2_row(p):
                    y1zT_s, b1_s, b2_s = l2t.pop(p)
                    a, b = 2 * p, 2 * p + 1
                    for m in range(2):
                        # chunk m of both imgs: N = [a w_out 256 | b w_out
                        # 256], bands via block-diagonal rhs over the pair
                        p2r = pssml.tile([128, 512], F32, tag="ps",
                                            name=f"p2r_{p}_{m}")
                        msl = slice(m * 128, (m + 1) * 128)
                        osl = slice(256 + m * 128, 256 + (m + 1) * 128)
                        mm(p2r[:], b1_s[:, msl], 'Be2_lo_bd2', True, False)
                        mm(p2r[:], b1_s[:, osl], 'Bo2_lo_bd2', False, False)
                        mm(p2r[:], b2_s[:, msl], 'Be2_hi_bd2', False, False)
                        mm(p2r[:], b2_s[:, osl], 'Bo2_hi_bd2', False, False)
                        mm(p2r[:, 0:256], y1zT_s[:, m * 128:(m + 1) * 128],
                           'M2_lo', False, True)
                        mm(p2r[:, 256:512],
                           y1zT_s[:, 256 + m * 128:256 + (m + 1) * 128],
                           'M2_lo', False, True)
                        za, zb = z1s_ap(a), z1s_ap(b)
                        if m == 0:
                            nc.scalar.copy(za[:, 0:256], p2r[:, 0:256])
                            nc.vector.tensor_copy(out=zb[:, 0:256],
                                                  in_=p2r[:, 256:512])
                        else:
                            nc.scalar.copy(za[:, 256:512], p2r[:, 0:256])
                            nc.vector.tensor_copy(out=zb[:, 256:512],
                                                  in_=p2r[:, 256:512])

                l2_col(0)
                for p in range(1, 8):
                    l2_col(p)
                    l2_row(p - 1)
                l2_row(7)

            # ===========================================================
            # Phase L1: z1 + yh0 bands -> out, software-pipelined
            # ===========================================================
            if True:
                l1t = {}

                def l1_col(img):
                    yh0t = yh0g[img // 4]
                    ib = (img % 4) * 1536
                    o_t = {o: yh0t[:, ib + o * 256:ib + (o + 1) * 256]
                           for o in range(6)}
                    z1_s = z1s_ap(img)
                    # phase A: y1 = band + lowpass, merged in w-polyphase
                    # layout [E(h 256) | O(h 256)]  (partitions = w')
                    p1a = pssml.tile([128, 512], F32, tag="ps",
                                        name=f"p1a_{img}")
                    y1_p = p1a[:]
                    mm(y1_p, o_t[0][:, 0::2], 'L1hi_w1r', True, False)
                    mm(y1_p, o_t[5][:, 0::2], 'L1hi_w2r', False, False)
                    mm(y1_p, o_t[0][:, 1::2], 'L1hi_w1i', False, False)
                    mm(y1_p, o_t[5][:, 1::2], 'L1hi_w2i', False, False)
                    mm(p1a[:, 0:256], z1_s[:, 0:256:2], 'Alo_a',
                       False, False)
                    mm(p1a[:, 0:256], z1_s[:, 256:512:2], 'Alo_b',
                       False, True)
                    mm(p1a[:, 256:512], z1_s[:, 1:256:2], 'Alo_a',
                       False, False)
                    mm(p1a[:, 256:512], z1_s[:, 257:512:2], 'Alo_b',
                       False, True)
                    y1_s = midpool.tile([128, 512], F16, tag="y1m",
                                        name=f"y1m_{img}")
                    nc.vector.tensor_copy(out=y1_s[:], in_=y1_p)

                    # phase B: y2b e|o [0:512)
                    p1b = pssml.tile([128, 512], F32, tag="ps",
                                        name=f"p1b_{img}")
                    y2b_p = p1b[:]
                    mm(y2b_p, o_t[2][:, 0::2], 'L1lo_w1r', True, False)
                    mm(y2b_p, o_t[3][:, 0::2], 'L1lo_w2r', False, False)
                    mm(y2b_p, o_t[2][:, 1::2], 'L1lo_w1i', False, False)
                    mm(y2b_p, o_t[3][:, 1::2], 'L1lo_w2i', False, False)
                    mm(y2b_p, o_t[1][:, 0::2], 'L1hi_w1r', False, False)
                    mm(y2b_p, o_t[4][:, 0::2], 'L1hi_w2r', False, False)
                    mm(y2b_p, o_t[1][:, 1::2], 'L1hi_w1i', False, False)
                    mm(y2b_p, o_t[4][:, 1::2], 'L1hi_w2i', False, True)
                    y2b1_s = midpool.tile([128, 512], F16, tag="y2b1",
                                          name=f"y2b1_{img}")
                    nc.vector.tensor_copy(out=y2b1_s[:], in_=y2b_p)
                    l1t[img] = (y1_s, y2b1_s)

                def l1_row(img):
                    y1_s, y2b1_s = l1t.pop(img)
                    # row stage -> out [256, 256] in two h-chunks; single
                    # store DMA per image ([a p] x <- p [a x])
                    p1r = pssml.tile([128, 512], F32, tag="ps",
                                        name=f"p1r_{img}")
                    ot = outpool.tile([128, 512], F16, tag="ot",
                                      name=f"ot_{img}")
                    for m in range(2):
                        oc = p1r[:, m * 256:(m + 1) * 256]
                        msl = slice(m * 128, (m + 1) * 128)
                        osl = slice(256 + m * 128, 256 + (m + 1) * 128)
                        mm(oc, y1_s[:, msl], 'Be1_lo', True, False)
                        mm(oc, y1_s[:, osl], 'Bo1_lo', False, False)
                        mm(oc, y2b1_s[:, msl], 'Be1_hi', False, False)
                        mm(oc, y2b1_s[:, osl], 'Bo1_hi', False, True)
                        if m == 0:
                            nc.scalar.copy(ot[:, 0:256], oc)
                        else:
                            nc.vector.tensor_copy(out=ot[:, 256:512], in_=oc)
                    nc.sync.dma_start(
                        out=out_d[img].rearrange("(a p) x -> p a x", a=2),
                        in_=ot.rearrange("p (a x) -> p a x", a=2))

                l1_col(0)
                for img in range(1, IMGS_PER_CORE):
                    l1_col(img)
                    l1_row(img - 1)
                l1_row(IMGS_PER_CORE - 1)

    split_excess_waits(nc)
    return nc


# ---------------------------------------------------------------------------
# Entry point
# ---------------------------------------------------------------------------
_NC_CACHE = []
_LAST_RESULT = []  # last BassKernelResults (exec_time_ns when BASS_TRACE=1)


def _axon_reset():
    try:
        import ctypes
        lib = ctypes.CDLL('/opt/axon/libaxon_pjrt.so')
        lib.axon_reset.restype = ctypes.c_int64
        lib.axon_reset()
    except Exception:
        pass


def _pack_pairs(yh, o_pair):
    """[16, 6, n, n, 2] -> [2n, 16*2n]: orientation pair partition-stacked,
    image-major columns with (w, ri) interleave."""
    a = yh[:, o_pair]                       # [16, 2, n, n, 2]
    n = a.shape[2]
    return np.ascontiguousarray(
        a.transpose(1, 2, 0, 3, 4).reshape(2 * n, 16 * 2 * n))


def kernel(yl, yh0, yh1, yh2, g0o, g1o, g0a, g0b, g1a, g1b):
    yl = np.asarray(yl, np.float16)
    yh0 = np.asarray(yh0, np.float16)
    yh1 = np.asarray(yh1, np.float16)
    yh2 = np.asarray(yh2, np.float16)
    assert yl.shape == (8, 16, 64, 64)

    mats = build_matrices(g0o, g1o, g0a, g0b, g1a, g1b)
    blobA0, blobA0R, blobA1, blobB = pack_blobs(mats)
    if not _NC_CACHE:
        _NC_CACHE.append(build_nc())
    nc = _NC_CACHE[0]

    in_maps = []
    for core in range(N_CORES):
        m = {"matsA0": blobA0, "matsA0R": blobA0R,
             "matsA1": blobA1, "matsB": blobB}
        m["z3b"] = np.ascontiguousarray(
            yl[core].transpose(1, 0, 2).reshape(64, 1024))
        m["lh3b"] = _pack_pairs(yh2[core], [0, 5])
        m["q3b"] = np.vstack([_pack_pairs(yh2[core], [2, 3]),
                              _pack_pairs(yh2[core], [1, 4])])
        m["lh2b"] = _pack_pairs(yh1[core], [0, 5])
        m["hl2b"] = _pack_pairs(yh1[core], [2, 3])
        m["hh2b"] = _pack_pairs(yh1[core], [1, 4])
        # yh0: [16, 6, 128, 256] -> [128, 16*1536] o-major per img, in
        # 4 groups of 4 imgs
        y0 = yh0[core].reshape(16, 6, 128, 256)
        y0 = y0.transpose(2, 0, 1, 3)       # [128, 16, 6, 256]
        for g in range(4):
            m[f"yh0b{g}"] = np.ascontiguousarray(
                y0[:, 4 * g:4 * g + 4].reshape(128, 4 * 1536))
        in_maps.append(m)

    try:
        res = run_bass_kernel_spmd(nc, in_maps, list(range(N_CORES)))
    except Exception as e:  # wedged exec unit: reset the axon device, retry
        if "UNAVAILABLE" not in str(e) and "unrecoverable" not in str(e):
            raise
        _axon_reset()
        res = run_bass_kernel_spmd(nc, in_maps, list(range(N_CORES)))
    _LAST_RESULT.clear()
    _LAST_RESULT.append(res)
    out = np.stack([res.results[i]["out"] for i in range(N_CORES)], axis=0)
    return np.ascontiguousarray(out.astype(np.float32))
